# revision 2
# baseline (speedup 1.0000x reference)
"""Trainium2 Bass kernel for DifferentiableCensus (3x3 soft census transform).

Full input x: [16, 3, 512, 512] f32. Output: same shape,
out = mean_{3x3 window, replicate pad} sigmoid(neighbor - center).

Sharding: pure data-parallel over batch: 8 cores x 2 batches; each core
processes 6 independent 512x512 images.

Algorithm (per image): symmetric-pair trick. With D4 = {E,S,SE,SW} and
t_d = sigmoid(shift_d(x) - x) (replicate-clamped), sigmoid(-u) = 1-sigmoid(u)
gives

  9*out[r,c] = 4.5 + sum_d t_d[r,c] - sum_d t_d[r-dr, c-dc]

exact everywhere provided t_d is evaluated on the replicate-extended grid
rows -1..511, cols -1..512 (clamped reads make every halo value correct by
construction). Only 4 sigmoid maps instead of 9.

Layout: overlapped strip layout. Each SBUF tensor holds 2 images on 64
partitions each. Partition p of an image covers rows 8p-1 .. 8p+8 (8 data
rows + 1 halo row each side) as free-dim "slots", cols -1..512 as padded
columns. All 9 stencil shifts are then pure free-dim offsets; no
cross-partition access anywhere. On-chip compute in fp16.

Engine distribution (the point of this version): every engine gets the work
it is uniquely good at, so all five run concurrently:
  - GpSimd (Pool): column-halo fills + f32->f16 conversion
  - Vector (DVE):  the 4 stencil diffs (fp16, 2x mode) + final affine
  - Scalar (ACT):  the 4 sigmoid maps
  - Tensor (PE):   the 8-term combine as +-identity matmuls accumulating
                   into PSUM (out9 = F - R), one 512-col slot per matmul
  - DMA:           HBM in/out
"""

import numpy as np

import concourse.bacc as bacc
import concourse.mybir as mybir
import concourse.tile as tile
from concourse import masks
from concourse.bass_utils import run_bass_kernel_spmd

F16 = mybir.dt.float16
F32 = mybir.dt.float32
SIG = mybir.ActivationFunctionType.Sigmoid
MUL = mybir.AluOpType.mult
ADD = mybir.AluOpType.add

N_CORES = 8
NIMG = 6
H = W = 512
S = 8          # data rows per partition
NS = S + 2     # slots incl. top/bottom halo rows
WP = W + 2     # idx m = image col m-1 (cols -1..512)
PPI = H // S   # partitions per image = 64
IPT = 128 // PPI  # images per tensor = 2
NT = NIMG // IPT  # tensors per core = 3
NR = NS - 1    # 9 stencil rows per partition (rows 8p-1 .. 8p+7)
HB = 4         # psum slots per half-pair (4 banks)


def _emit_pair(nc, pools, wp, wn, x_imgs, y_imgs):
    """Emit ops for IPT images sharing one 128-partition tensor set."""
    pio, pmid, ptmp, pps, pout = pools

    xf = pio.tile([128, NS, WP], F32, name="xf")
    for g, ximg in enumerate(x_imgs):
        B = g * PPI
        xs = ximg.rearrange("(p s) c -> p s c", s=S)  # [64, 8, 512]
        # main slots 1..8 = rows 8p..8p+7
        nc.sync.dma_start(out=xf[B : B + PPI, 1 : S + 1, 1 : W + 1], in_=xs)
        # slot 0 = row 8p-1: for p>=1 it is row 8(p-1)+7
        nc.sync.dma_start(
            out=xf[B + 1 : B + PPI, 0:1, 1 : W + 1], in_=xs[0 : PPI - 1, S - 1 : S, :]
        )
        # partition 0 slot 0 = row -1 := row 0 (replicate)
        nc.sync.dma_start(out=xf[B : B + 1, 0:1, 1 : W + 1], in_=xs[0:1, 0:1, :])
        # slot 9 = row 8p+8: for p<=62 it is row 8(p+1)
        nc.sync.dma_start(
            out=xf[B : B + PPI - 1, S + 1 : S + 2, 1 : W + 1], in_=xs[1:PPI, 0:1, :]
        )
        # partition 63 slot 9 = row 512 := row 511 (replicate)
        nc.sync.dma_start(
            out=xf[B + PPI - 1 : B + PPI, S + 1 : S + 2, 1 : W + 1],
            in_=xs[PPI - 1 : PPI, S - 1 : S, :],
        )
    # column halos: idx 0 := col 0 (idx 1), idx 513 := col 511 (idx 512)
    nc.gpsimd.tensor_copy(out=xf[:, :, 0:1], in_=xf[:, :, 1:2])
    nc.gpsimd.tensor_copy(out=xf[:, :, W + 1 : W + 2], in_=xf[:, :, W : W + 1])

    # convert to fp16 (GpSimd so DVE stays free for the diffs)
    xh = pmid.tile([128, NS, WP], F16, name="xh")
    nc.gpsimd.tensor_copy(out=xh[:], in_=xf[:])

    # ---- diffs (DVE) + sigmoids (ACT) on the extended grid ----
    # t_d[row, col] computed for rows -1..511 (slots 0..8); col ranges per map.
    tE = pmid.tile([128, NR, WP], F16, name="tE")
    tS = pmid.tile([128, NR, WP], F16, name="tS")
    tSE = pmid.tile([128, NR, WP], F16, name="tSE")
    tSW = pmid.tile([128, NR, WP], F16, name="tSW")

    # E: idx 0..512 (cols -1..511): d = x[r, m+1] - x[r, m]
    dE = ptmp.tile([128, NR, WP], F16, name="dE", tag="d")
    nc.vector.tensor_sub(
        out=dE[:, :, 0 : W + 1], in0=xh[:, 0:NR, 1 : W + 2], in1=xh[:, 0:NR, 0 : W + 1]
    )
    nc.scalar.activation(out=tE[:, :, 0 : W + 1], in_=dE[:, :, 0 : W + 1], func=SIG)

    # S: idx 1..512 (cols 0..511): d = x[r+1, m] - x[r, m]
    dS = ptmp.tile([128, NR, WP], F16, name="dS", tag="d")
    nc.vector.tensor_sub(
        out=dS[:, :, 1 : W + 1], in0=xh[:, 1:NS, 1 : W + 1], in1=xh[:, 0:NR, 1 : W + 1]
    )
    nc.scalar.activation(out=tS[:, :, 1 : W + 1], in_=dS[:, :, 1 : W + 1], func=SIG)

    # SE: idx 0..512 (cols -1..511): d = x[r+1, m+1] - x[r, m]
    dSE = ptmp.tile([128, NR, WP], F16, name="dSE", tag="d")
    nc.vector.tensor_sub(
        out=dSE[:, :, 0 : W + 1],
        in0=xh[:, 1:NS, 1 : W + 2],
        in1=xh[:, 0:NR, 0 : W + 1],
    )
    nc.scalar.activation(out=tSE[:, :, 0 : W + 1], in_=dSE[:, :, 0 : W + 1], func=SIG)

    # SW: idx 1..513 (cols 0..512): d = x[r+1, m-1] - x[r, m]
    dSW = ptmp.tile([128, NR, WP], F16, name="dSW", tag="d")
    nc.vector.tensor_sub(
        out=dSW[:, :, 1 : W + 2],
        in0=xh[:, 1:NS, 0 : W + 1],
        in1=xh[:, 0:NR, 1 : W + 2],
    )
    nc.scalar.activation(out=tSW[:, :, 1 : W + 2], in_=dSW[:, :, 1 : W + 2], func=SIG)

    # ---- combine on the Tensor engine ----
    # For output row r = 8p+s (s=0..7), image col c (t-map col idx c+1):
    #   forward terms (+1): t_d at slot s+1:
    #     E/S/SE/SW all at [s+1, idx c+1]
    #   reverse terms (-1): t_d[r-dr, c-dc]:
    #     E:  t_E[r, c-1]   -> slot s+1, idx c
    #     S:  t_S[r-1, c]   -> slot s,   idx c+1
    #     SE: t_SE[r-1,c-1] -> slot s,   idx c
    #     SW: t_SW[r-1,c+1] -> slot s,   idx c+2
    # PSUM accumulates F - R via identity weights (wp=+I, wn=-I), one
    # 512-col matmul per term per output slot; 4 slots per PSUM tile.
    for half in range(2):
        ps = pps.tile([128, HB, W], F32, name="ps")
        for si in range(HB):
            s = half * HB + si
            bank = ps[:, si, :]
            terms = (
                (wp, tE[:, s + 1, 1 : W + 1]),
                (wp, tS[:, s + 1, 1 : W + 1]),
                (wp, tSE[:, s + 1, 1 : W + 1]),
                (wp, tSW[:, s + 1, 1 : W + 1]),
                (wn, tE[:, s + 1, 0:W]),
                (wn, tS[:, s, 1 : W + 1]),
                (wn, tSE[:, s, 0:W]),
                (wn, tSW[:, s, 2 : W + 2]),
            )
            for j, (w, rhs) in enumerate(terms):
                nc.tensor.matmul(
                    out=bank, lhsT=w, rhs=rhs, start=(j == 0), stop=(j == 7)
                )
        # out = (F - R)/9 + (4.5/9 = 0.5)
        of32 = pout.tile([128, HB, W], F32, name="of32")
        nc.vector.tensor_scalar(
            out=of32[:], in0=ps[:], scalar1=1.0 / 9.0, scalar2=0.5, op0=MUL, op1=ADD
        )
        for g, yimg in enumerate(y_imgs):
            B = g * PPI
            ys = yimg.rearrange("(p s) c -> p s c", s=S)
            nc.sync.dma_start(
                out=ys[:, half * HB : (half + 1) * HB, :],
                in_=of32[B : B + PPI, :, :],
            )


_CACHED_NC = None


def _build():
    global _CACHED_NC
    if _CACHED_NC is not None:
        return _CACHED_NC
    nc = bacc.Bacc("TRN2", target_bir_lowering=False, debug=False)
    x = nc.dram_tensor("x", [NIMG, H, W], F32, kind="ExternalInput")
    y = nc.dram_tensor("y", [NIMG, H, W], F32, kind="ExternalOutput")
    with tile.TileContext(nc) as tc:
        with (
            tc.tile_pool(name="pw", bufs=1) as pw,
            tc.tile_pool(name="pio", bufs=2) as pio,
            tc.tile_pool(name="pmid", bufs=2) as pmid,
            tc.tile_pool(name="ptmp", bufs=2) as ptmp,
            tc.psum_pool(name="pps", bufs=2) as pps,
            tc.tile_pool(name="pout", bufs=2) as pout,
        ):
            wp = pw.tile([128, 128], F16, name="wp")
            wn = pw.tile([128, 128], F16, name="wn")
            masks.make_identity(nc, wp[:])
            nc.gpsimd.memset(wn[:], 0.0)
            nc.gpsimd.affine_select(
                out=wn[:],
                in_=wn[:],
                compare_op=mybir.AluOpType.not_equal,
                fill=-1.0,
                base=0,
                pattern=[[-1, 128]],
                channel_multiplier=1,
            )
            pools = (pio, pmid, ptmp, pps, pout)
            for t in range(NT):
                imgs = list(range(t * IPT, (t + 1) * IPT))
                _emit_pair(
                    nc,
                    pools,
                    wp[:],
                    wn[:],
                    [x.ap()[i] for i in imgs],
                    [y.ap()[i] for i in imgs],
                )
    nc.compile()
    _CACHED_NC = nc
    return nc


def kernel(x: np.ndarray) -> np.ndarray:
    assert x.shape == (16, 3, 512, 512) and x.dtype == np.float32
    nc = _build()
    xs = x.reshape(N_CORES, NIMG, H, W)
    in_maps = [{"x": np.ascontiguousarray(xs[i])} for i in range(N_CORES)]
    res = run_bass_kernel_spmd(nc, in_maps, core_ids=list(range(N_CORES)))
    out = np.stack([res.results[i]["y"] for i in range(N_CORES)])
    return out.reshape(16, 3, 512, 512)


# revision 4
# speedup vs baseline: 1.3296x; 1.3296x over previous
"""Trainium2 Bass kernel for DifferentiableCensus (3x3 soft census transform).

Full input x: [16, 3, 512, 512] f32. Output: same shape,
out = mean_{3x3 window, replicate pad} sigmoid(neighbor - center).

Sharding: pure data-parallel over batch: 8 cores x 2 batches; each core
processes 6 independent 512x512 images.

Algorithm (per image): symmetric-pair trick. With D4 = {E,S,SE,SW} and
t_d = sigmoid(shift_d(x) - x) (replicate-clamped), sigmoid(-u) = 1-sigmoid(u)
gives

  9*out[r,c] = 4.5 + sum_d t_d[r,c] - sum_d t_d[r-dr, c-dc]

exact everywhere provided t_d is evaluated on the replicate-extended grid
(clamped reads make every halo value correct by construction). Only 4
sigmoid maps instead of 9.

Layout: overlapped strip layout. Each SBUF tensor holds 2 images on 64
partitions each. Partition p of an image covers rows 8p-1 .. 8p+8 (8 data
rows + 1 halo row each side) as free-dim "slots", cols -1..512 as padded
columns. All stencil shifts are then pure free-dim offsets.

Engine distribution:
  - DMA (Pool SWDGE): ONE casting DMA per pair loads the 8 data rows per
    partition as fp16 straight from f32 HBM (software DGE casts in-flight,
    so no on-chip conversion pass and no f32 SBUF tile at all)
  - DMA (HWDGE): two SBUF->SBUF partition-shifted copies build the row
    halos on-chip (no duplicate HBM reads)
  - GpSimd:        per-image edge-row replicate fixes + column halos
  - Vector (DVE):  the 4 stencil diffs (fp16, 2x mode) + final affine
  - Scalar (ACT):  the 4 sigmoid maps
  - Tensor (PE):   the 8-term combine as +-identity matmuls accumulating
                   into PSUM (out9 = F - R), one 512-col slot per matmul
"""

import numpy as np

import concourse.bacc as bacc
import concourse.mybir as mybir
import concourse.tile as tile
from concourse import masks
from concourse.bass_utils import run_bass_kernel_spmd

F16 = mybir.dt.float16
F32 = mybir.dt.float32
SIG = mybir.ActivationFunctionType.Sigmoid
MUL = mybir.AluOpType.mult
ADD = mybir.AluOpType.add

N_CORES = 8
NIMG = 6
H = W = 512
S = 8          # data rows per partition
NS = S + 2     # slots incl. top/bottom halo rows
WP = W + 2     # idx m = image col m-1 (cols -1..512)
PPI = H // S   # partitions per image = 64
IPT = 128 // PPI  # images per tensor = 2
NT = NIMG // IPT  # tensors per core = 3
NR = NS - 1    # 9 stencil rows per partition (rows 8p-1 .. 8p+7)
HB = 4         # psum slots per half-pair (4 banks)


def _emit_pair(nc, pools, wp, wn, xpair, y_imgs):
    """Emit ops for IPT images sharing one 128-partition tensor set.

    xpair: DRAM view [2*H, W] of this pair's two images (contiguous).
    """
    pmid, ptmp, pps, pout = pools

    xh = pmid.tile([128, NS, WP], F16, name="xh")

    # main slots 1..8 = rows 8p..8p+7 of the flattened 1024-row pair,
    # cast f32 -> fp16 in-flight (software DGE on Pool).
    xs = xpair.rearrange("(p s) c -> p s c", s=S)  # [128, 8, 512]
    nc.gpsimd.dma_start(out=xh[:, 1 : S + 1, 1 : W + 1], in_=xs)

    # row halos on-chip (partition-shifted SBUF->SBUF copies):
    # slot 0[p] = row 8p-1 = slot 8[p-1];  slot 9[p] = row 8p+8 = slot 1[p+1]
    nc.sync.dma_start(out=xh[1:128, 0:1, 1 : W + 1], in_=xh[0:127, S : S + 1, 1 : W + 1])
    nc.sync.dma_start(out=xh[0:127, S + 1 : S + 2, 1 : W + 1], in_=xh[1:128, 1:2, 1 : W + 1])
    # replicate fixes at image edges: top row of each image (p=0,64: slot0 :=
    # row 0 = slot1), bottom row (p=63,127: slot9 := row 511 = slot8).
    # DMAs, not tensor ops: the BIR verifier rejects compute-engine accesses
    # at unaligned partition starts.
    for g in range(IPT):
        B = g * PPI
        nc.sync.dma_start(
            out=xh[B : B + 1, 0:1, 1 : W + 1], in_=xh[B : B + 1, 1:2, 1 : W + 1]
        )
        nc.sync.dma_start(
            out=xh[B + PPI - 1 : B + PPI, S + 1 : S + 2, 1 : W + 1],
            in_=xh[B + PPI - 1 : B + PPI, S : S + 1, 1 : W + 1],
        )
    # column halos: idx 0 := col 0 (idx 1), idx 513 := col 511 (idx 512)
    nc.gpsimd.tensor_copy(out=xh[:, :, 0:1], in_=xh[:, :, 1:2])
    nc.gpsimd.tensor_copy(out=xh[:, :, W + 1 : W + 2], in_=xh[:, :, W : W + 1])

    # ---- diffs (DVE) + sigmoids (ACT) on the extended grid ----
    # t_d[row, col] for slots 0..8 (rows -1..511); E only needs slots 1..8.
    tE = pmid.tile([128, NR, WP], F16, name="tE")
    tS = pmid.tile([128, NR, WP], F16, name="tS")
    tSE = pmid.tile([128, NR, WP], F16, name="tSE")
    tSW = pmid.tile([128, NR, WP], F16, name="tSW")

    # E: slots 1..8, idx 0..512 (cols -1..511): d = x[r, m+1] - x[r, m]
    dE = ptmp.tile([128, NR, WP], F16, name="dE", tag="d", bufs=4)
    nc.vector.tensor_sub(
        out=dE[:, 1:NR, 0 : W + 1],
        in0=xh[:, 1:NR, 1 : W + 2],
        in1=xh[:, 1:NR, 0 : W + 1],
    )
    nc.scalar.activation(out=tE[:, 1:NR, 0 : W + 1], in_=dE[:, 1:NR, 0 : W + 1], func=SIG)

    # S: slots 0..8, idx 1..512 (cols 0..511): d = x[r+1, m] - x[r, m]
    dS = ptmp.tile([128, NR, WP], F16, name="dS", tag="d", bufs=4)
    nc.vector.tensor_sub(
        out=dS[:, :, 1 : W + 1], in0=xh[:, 1:NS, 1 : W + 1], in1=xh[:, 0:NR, 1 : W + 1]
    )
    nc.scalar.activation(out=tS[:, :, 1 : W + 1], in_=dS[:, :, 1 : W + 1], func=SIG)

    # SE: slots 0..8, idx 0..512 (cols -1..511): d = x[r+1, m+1] - x[r, m]
    dSE = ptmp.tile([128, NR, WP], F16, name="dSE", tag="d", bufs=4)
    nc.vector.tensor_sub(
        out=dSE[:, :, 0 : W + 1],
        in0=xh[:, 1:NS, 1 : W + 2],
        in1=xh[:, 0:NR, 0 : W + 1],
    )
    nc.scalar.activation(out=tSE[:, :, 0 : W + 1], in_=dSE[:, :, 0 : W + 1], func=SIG)

    # SW: slots 0..8, idx 1..513 (cols 0..512): d = x[r+1, m-1] - x[r, m]
    dSW = ptmp.tile([128, NR, WP], F16, name="dSW", tag="d", bufs=4)
    nc.vector.tensor_sub(
        out=dSW[:, :, 1 : W + 2],
        in0=xh[:, 1:NS, 0 : W + 1],
        in1=xh[:, 0:NR, 1 : W + 2],
    )
    nc.scalar.activation(out=tSW[:, :, 1 : W + 2], in_=dSW[:, :, 1 : W + 2], func=SIG)

    # ---- combine on the Tensor engine ----
    # For output row r = 8p+s (s=0..7), image col c (t-map col idx c+1):
    #   forward (+1): all t_d at [slot s+1, idx c+1]
    #   reverse (-1): E [s+1, c], S [s, c+1], SE [s, c], SW [s, c+2]
    # PSUM accumulates F - R via identity weights (wp=+I, wn=-I), one
    # 512-col matmul per term per output slot; 4 slots per PSUM tile.
    for half in range(2):
        ps = pps.tile([128, HB, W], F32, name="ps")
        for si in range(HB):
            s = half * HB + si
            bank = ps[:, si, :]
            terms = (
                (wp, tE[:, s + 1, 1 : W + 1]),
                (wp, tS[:, s + 1, 1 : W + 1]),
                (wp, tSE[:, s + 1, 1 : W + 1]),
                (wp, tSW[:, s + 1, 1 : W + 1]),
                (wn, tE[:, s + 1, 0:W]),
                (wn, tS[:, s, 1 : W + 1]),
                (wn, tSE[:, s, 0:W]),
                (wn, tSW[:, s, 2 : W + 2]),
            )
            for j, (w, rhs) in enumerate(terms):
                nc.tensor.matmul(
                    out=bank, lhsT=w, rhs=rhs, start=(j == 0), stop=(j == 7)
                )
        # out = (F - R)/9 + (4.5/9 = 0.5)
        of32 = pout.tile([128, HB, W], F32, name="of32")
        nc.vector.tensor_scalar(
            out=of32[:], in0=ps[:], scalar1=1.0 / 9.0, scalar2=0.5, op0=MUL, op1=ADD
        )
        for g, yimg in enumerate(y_imgs):
            B = g * PPI
            ys = yimg.rearrange("(p s) c -> p s c", s=S)
            nc.sync.dma_start(
                out=ys[:, half * HB : (half + 1) * HB, :],
                in_=of32[B : B + PPI, :, :],
            )


_CACHED_NC = None


def _build():
    global _CACHED_NC
    if _CACHED_NC is not None:
        return _CACHED_NC
    nc = bacc.Bacc("TRN2", target_bir_lowering=False, debug=False)
    x = nc.dram_tensor("x", [NIMG, H, W], F32, kind="ExternalInput")
    y = nc.dram_tensor("y", [NIMG, H, W], F32, kind="ExternalOutput")
    xflat = x.ap().rearrange("i h c -> (i h) c")  # [3072, 512]
    with tile.TileContext(nc) as tc:
        with (
            tc.tile_pool(name="pw", bufs=1) as pw,
            tc.tile_pool(name="pmid", bufs=2) as pmid,
            tc.tile_pool(name="ptmp", bufs=4) as ptmp,
            tc.psum_pool(name="pps", bufs=2) as pps,
            tc.tile_pool(name="pout", bufs=2) as pout,
        ):
            wp = pw.tile([128, 128], F16, name="wp")
            wn = pw.tile([128, 128], F16, name="wn")
            masks.make_identity(nc, wp[:])
            nc.gpsimd.memset(wn[:], 0.0)
            nc.gpsimd.affine_select(
                out=wn[:],
                in_=wn[:],
                compare_op=mybir.AluOpType.not_equal,
                fill=-1.0,
                base=0,
                pattern=[[-1, 128]],
                channel_multiplier=1,
            )
            pools = (pmid, ptmp, pps, pout)
            for t in range(NT):
                imgs = list(range(t * IPT, (t + 1) * IPT))
                _emit_pair(
                    nc,
                    pools,
                    wp[:],
                    wn[:],
                    xflat[t * IPT * H : (t + 1) * IPT * H, :],
                    [y.ap()[i] for i in imgs],
                )
    nc.compile()
    _CACHED_NC = nc
    return nc


def kernel(x: np.ndarray) -> np.ndarray:
    assert x.shape == (16, 3, 512, 512) and x.dtype == np.float32
    nc = _build()
    xs = x.reshape(N_CORES, NIMG, H, W)
    in_maps = [{"x": np.ascontiguousarray(xs[i])} for i in range(N_CORES)]
    res = run_bass_kernel_spmd(nc, in_maps, core_ids=list(range(N_CORES)))
    out = np.stack([res.results[i]["y"] for i in range(N_CORES)])
    return out.reshape(16, 3, 512, 512)
